# revision 9
# baseline (speedup 1.0000x reference)
"""LSS (lift-splat-shoot) BEV transform kernel for 8 trn2 NeuronCores.

Pipeline (per core, SPMD single NEFF):
  stage A: feat = w_depth @ x + b  (per-pixel 1x1 conv as matmul)
  stage B: softmax over 41 depth bins, cfeat = feat[41:169]
  stage C: dvalid = depth * validity-mask (host-computed mask)
  stage D: h-contraction  T[col,d,c] = sum_h dvalid[col,h,d]*cfeat[col,h,c]
           (valid because voxel rank is h-invariant per (cam,d,w) for this
            camera geometry; host verifies and splits h-groups otherwise)
  stage E: route T rows by owning core via indirect-scatter + AllToAll
  stage F: segment-sum routed rows with one-hot matmuls into per-piece rows
Host: geometry/rank computation, routing tables, one-hot R matrices, and
final piece->voxel accumulation + layout transpose.
"""

import math
import os

import numpy as np

# ---------------- problem constants (hardcoded; must match reference) -----
OGF_H, OGF_W = 256, 704
DOWNSAMPLE = 16
FH, FW = OGF_H // DOWNSAMPLE, OGF_W // DOWNSAMPLE  # 16, 44
D_BINS = 41
C_TRANS = 128
NX, NY, NZ = 128, 128, 1
DX = np.array([0.8, 0.8, 20.0], np.float32)
BX = np.array([-50.8, -50.8, 0.0], np.float32)
NCORES = 8
CIN = 512
NSEG = NX * NY * NZ  # 16384 (B=1)

LAST_EXEC_NS = None
LAST_RESULTS = None


def _make_frustum():
    ds = np.arange(4.0, 45.0, 1.0, dtype=np.float32)[:, None, None] * np.ones(
        (1, FH, FW), np.float32
    )
    xs = np.linspace(0.0, OGF_W - 1.0, FW, dtype=np.float32)[None, None, :] * np.ones(
        (D_BINS, FH, 1), np.float32
    )
    ys = np.linspace(0.0, OGF_H - 1.0, FH, dtype=np.float32)[None, :, None] * np.ones(
        (D_BINS, 1, FW), np.float32
    )
    return np.stack([xs, ys, ds], axis=-1)  # (D, H, W, 3)


def _geometry(rots, trans, intrins, post_rots, post_trans):
    """Replicates reference get_geometry in numpy float32.
    Returns gi (B,N,D,H,W,3) int32 voxel indices and valid mask."""
    frustum = _make_frustum()
    inv_post = np.linalg.inv(post_rots.astype(np.float32)).astype(np.float32)
    inv_intr = np.linalg.inv(intrins.astype(np.float32)).astype(np.float32)
    pts = frustum[None, None] - post_trans[:, :, None, None, None, :]
    pts = np.einsum("bnij,bndhwj->bndhwi", inv_post, pts).astype(np.float32)
    pts = np.concatenate([pts[..., :2] * pts[..., 2:3], pts[..., 2:3]], axis=-1)
    combine = np.einsum("bnij,bnjk->bnik", rots, inv_intr).astype(np.float32)
    geom = (
        np.einsum("bnij,bndhwj->bndhwi", combine, pts).astype(np.float32)
        + trans[:, :, None, None, None, :]
    ).astype(np.float32)
    gi = ((geom - (BX - DX / 2.0)) / DX).astype(np.int32)
    valid = (
        (gi[..., 0] >= 0)
        & (gi[..., 0] < NX)
        & (gi[..., 1] >= 0)
        & (gi[..., 1] < NY)
        & (gi[..., 2] >= 0)
        & (gi[..., 2] < NZ)
    )
    return gi, valid


def _build_columns(gi, valid):
    """Build h-collapsed columns. Each column = (cam n, pixel w, h-mask) s.t.
    for every d the valid members share one voxel rank.
    Returns list of dicts with n, w, hmask[FH], rank[d] (int32, -1 invalid),
    mask[d, h] float32."""
    # rank per point (valid points only meaningful)
    rank = gi[..., 0].astype(np.int64) * (NY * NZ) + gi[..., 1] * NZ + gi[..., 2]
    cols = []
    B, N = gi.shape[0], gi.shape[1]
    assert B == 1
    for n in range(N):
        for w in range(FW):
            r = rank[0, n, :, :, w]  # (D, H)
            v = valid[0, n, :, :, w]  # (D, H)
            # greedy group h's so that within a group every d has <=1 rank
            groups = []  # list of (hlist, rank_per_d array)
            for h in range(FH):
                placed = False
                for hl, rpd in groups:
                    ok = True
                    for d in range(D_BINS):
                        if v[d, h] and rpd[d] >= 0 and rpd[d] != r[d, h]:
                            ok = False
                            break
                    if ok:
                        hl.append(h)
                        for d in range(D_BINS):
                            if v[d, h]:
                                rpd[d] = r[d, h]
                        placed = True
                        break
                if not placed:
                    rpd = np.full(D_BINS, -1, np.int64)
                    for d in range(D_BINS):
                        if v[d, h]:
                            rpd[d] = r[d, h]
                    groups.append(([h], rpd))
            for hl, rpd in groups:
                mask = np.zeros((D_BINS, FH), np.float32)
                for h in hl:
                    mask[:, h] = v[:, h].astype(np.float32)
                cols.append(dict(n=n, w=w, rank=rpd, mask=mask))
    return cols


def _fast_columns(gi, valid):
    """Fast path: verify rank is h-invariant per (n,d,w) among valid h's.
    Returns columns list or None if the property fails."""
    rank = gi[..., 0].astype(np.int64) * (NY * NZ) + gi[..., 1] * NZ + gi[..., 2]
    r = rank[0]  # (N, D, H, W)
    v = valid[0]
    rv = np.where(v, r, -1)
    mx = rv.max(axis=2)  # (N, D, W)
    # conflict if any valid h has rank != max
    conflict = (v & (rv != mx[:, :, None, :])).any(axis=2)  # (N, D, W)
    if conflict.any():
        return None
    cols = []
    for n in range(r.shape[0]):
        for w in range(FW):
            rpd = mx[n, :, w].copy()  # -1 where no valid h
            mask = v[n, :, :, w].astype(np.float32)  # (D, H)
            cols.append(dict(n=n, w=w, rank=rpd, mask=mask))
    return cols


class _Plan:
    pass


def _make_plan(inputs):
    x = np.asarray(inputs["x"], np.float32)
    gi, valid = _geometry(
        np.asarray(inputs["rots"], np.float32),
        np.asarray(inputs["trans"], np.float32),
        np.asarray(inputs["intrins"], np.float32),
        np.asarray(inputs["post_rots"], np.float32),
        np.asarray(inputs["post_trans"], np.float32),
    )
    cols = _fast_columns(gi, valid)
    if cols is None:
        cols = _build_columns(gi, valid)

    # pad column count to multiple of 48 (8 cores x 3 cols/tile x 2/group)
    while len(cols) % 48 != 0:
        cols.append(
            dict(n=0, w=0, rank=np.full(D_BINS, -1, np.int64),
                 mask=np.zeros((D_BINS, FH), np.float32))
        )
    NCOLS = len(cols)
    CPC = NCOLS // NCORES          # columns per core (multiple of 6)
    GROUPS = CPC // 2              # stage-D psum groups of 2 columns
    TILES = CPC // 3               # 128-partition tiles (3 cols per tile)
    PX = TILES * 128               # padded pixel partitions per core

    # ---- sortless slot scheme ----
    # source slot space: srcslot = g*105 + p, p = 64*q + d (q=a%2, g=a//2)
    # AllToAll: dest t receives slice [t*SH,(t+1)*SH) of every source's slots
    SLOTS_REAL = 105 * GROUPS
    SH0 = (SLOTS_REAL + NCORES - 1) // NCORES
    SLOTS = ((NCORES * SH0 + 127) // 128) * 128
    SH = SLOTS // NCORES
    NCHUNK = SLOTS // 128

    # rank per (gcol, d); -1 = no contribution
    rank_of = np.full((NCOLS, D_BINS), -1, np.int64)
    for g, c in enumerate(cols):
        m_any = c["mask"].any(axis=1)
        rk = np.asarray(c["rank"])
        rank_of[g] = np.where(m_any & (rk >= 0), rk, -1)

    rmat = np.zeros((NCORES, 128, NCHUNK * 128), np.float32)
    piece_row = [[] for _ in range(NCORES)]
    piece_rank = [[] for _ in range(NCORES)]
    for t in range(NCORES):
        for j in range(NCHUNK):
            run_of = {}
            for p_loc in range(128):
                i = j * 128 + p_loc          # dest slot
                srcs = i // SH
                srcslot = t * SH + (i - srcs * SH)
                if srcslot >= SLOTS_REAL:
                    continue
                p = srcslot % 105
                g = srcslot // 105
                if 41 <= p < 64:
                    continue
                q = 1 if p >= 64 else 0
                d = p - 64 * q
                if d >= D_BINS:
                    continue
                gcol = srcs * CPC + g * 2 + q
                rk = rank_of[gcol, d]
                if rk < 0:
                    continue
                if rk not in run_of:
                    run_of[rk] = len(run_of)
                    piece_row[t].append(j * 128 + run_of[rk])
                    piece_rank[t].append(rk)
                rmat[t, p_loc, j * 128 + run_of[rk]] = 1.0

    # ---- x_loc, mask, weights ----
    xin = np.zeros((NCORES, 4, 128, PX), np.float32)
    mk = np.zeros((NCORES, TILES, 128, D_BINS), np.float32)
    for cidx in range(NCORES):
        for a in range(CPC):
            c = cols[cidx * CPC + a]
            xcol = x[0, c["n"], :, :, c["w"]]  # (512, FH)
            base = (a // 3) * 128 + (a % 3) * 32
            xin[cidx, :, :, base:base + FH] = xcol.reshape(4, 128, FH)
            tt, po = a // 3, (a % 3) * 32
            mk[cidx, tt, po:po + FH, :] = c["mask"].T  # (FH, D)

    w_depth = np.asarray(inputs["w_depth"], np.float32)  # (169, 512)
    wt = np.ascontiguousarray(
        w_depth.T.reshape(4, 128, D_BINS + C_TRANS)
    )  # wt[k] = w_depth[:, 128k:128k+128].T
    bv = np.asarray(inputs["b_depth"], np.float32).reshape(1, D_BINS + C_TRANS)

    pl = _Plan()
    pl.NCOLS, pl.CPC, pl.GROUPS, pl.PX, pl.TILES = NCOLS, CPC, GROUPS, PX, TILES
    pl.NCHUNK, pl.SH, pl.SLOTS, pl.SLOTS_REAL = NCHUNK, SH, SLOTS, SLOTS_REAL
    pl.rmat = rmat
    pl.piece_row = [np.array(p, np.int64) for p in piece_row]
    pl.piece_rank = [np.array(p, np.int64) for p in piece_rank]
    pl.xin, pl.mk, pl.wt, pl.bv = xin, mk, wt, bv
    return pl


# ------------------------- device program ---------------------------------

def _build_program(pl):
    import concourse.bass as bass
    import concourse.mybir as mybir
    import concourse.tile as tile
    from concourse import bacc

    f32 = mybir.dt.float32
    i32 = mybir.dt.int32
    AX = mybir.AxisListType.X
    OP = mybir.AluOpType
    ACT = mybir.ActivationFunctionType

    CPC, GROUPS, PX, TILES = pl.CPC, pl.GROUPS, pl.PX, pl.TILES
    NCHUNK, SH = pl.NCHUNK, pl.SH
    NO = D_BINS + C_TRANS  # 169

    nc = bacc.Bacc("TRN2", target_bir_lowering=False, debug=False,
                   num_devices=NCORES)

    xin = nc.dram_tensor("xin", [4, 128, PX], f32, kind="ExternalInput")
    wt = nc.dram_tensor("wt", [4, 128, NO], f32, kind="ExternalInput")
    bv = nc.dram_tensor("bv", [1, NO], f32, kind="ExternalInput")
    mk = nc.dram_tensor("mk", [TILES, 128, D_BINS], f32, kind="ExternalInput")
    SLOTS, SLOTS_REAL = pl.SLOTS, pl.SLOTS_REAL
    rmat = nc.dram_tensor("rmat", [128, NCHUNK * 128], f32, kind="ExternalInput")
    out2 = nc.dram_tensor("out2", [NCHUNK * 128, 128], f32, kind="ExternalOutput")
    debug = bool(int(os.environ.get("KERNEL_DEBUG", "0")))
    if debug:
        dbg_t = nc.dram_tensor("dbg_t", [105, GROUPS * 128], f32, kind="ExternalOutput")
        dbg_ain = nc.dram_tensor("dbg_ain", [SLOTS, 128], f32, kind="ExternalOutput")
        dbg_aout = nc.dram_tensor("dbg_aout", [SLOTS, 128], f32, kind="ExternalOutput")
        dbg_u = nc.dram_tensor("dbg_u", [128, NCHUNK * 128], f32, kind="ExternalOutput")

    with tile.TileContext(nc) as tc:
        with (
            tc.tile_pool(name="const", bufs=1) as cpool,
            tc.tile_pool(name="work", bufs=1) as wpool,
            tc.tile_pool(name="stats", bufs=4) as spool,
            tc.tile_pool(name="pf", bufs=2, space="PSUM") as pfp,
            tc.tile_pool(name="pt", bufs=4, space="PSUM") as ptp,
            tc.tile_pool(name="ps", bufs=2, space="PSUM") as psp,
            tc.tile_pool(name="dram", bufs=1, space="DRAM") as dpool,
        ):
            xbuf = cpool.tile([128, 4, PX], f32)
            wbuf = cpool.tile([128, 4, NO], f32)
            bbuf = cpool.tile([1, NO], f32)
            mbuf = cpool.tile([128, TILES, D_BINS], f32)
            rbuf = cpool.tile([128, NCHUNK, 128], f32)
            onesb = cpool.tile([1, PX], f32)

            for k in range(4):
                nc.sync.dma_start(out=xbuf[:, k, :], in_=xin[k])
                nc.sync.dma_start(out=wbuf[:, k, :], in_=wt[k])
            nc.sync.dma_start(out=bbuf[:], in_=bv[:])
            for t in range(TILES):
                nc.sync.dma_start(out=mbuf[:, t, :], in_=mk[t])
            nc.sync.dma_start(
                out=rbuf[:].rearrange("p j c -> p (j c)"), in_=rmat[:]
            )
            nc.vector.memset(onesb[:], 1.0)

            dvalb = wpool.tile([128, TILES, D_BINS], f32)
            cfb = wpool.tile([128, TILES, C_TRANS], f32)
            tbuf = wpool.tile([105, GROUPS, 128], f32)
            zrows = cpool.tile([32, 128], f32)
            nc.vector.memset(tbuf[32:64, :, :], 0.0)
            nc.vector.memset(zrows[:], 0.0)

            for t in range(TILES):
                Pt = 128
                pf = pfp.tile([128, NO], f32)
                for k in range(4):
                    nc.tensor.matmul(
                        pf[:Pt],
                        lhsT=xbuf[:, k, t * 128:t * 128 + Pt],
                        rhs=wbuf[:, k, :],
                        start=(k == 0),
                        stop=False,
                    )
                nc.tensor.matmul(
                    pf[:Pt],
                    lhsT=onesb[:1, t * 128:t * 128 + Pt],
                    rhs=bbuf[:1, :],
                    start=False,
                    stop=True,
                )
                mx = spool.tile([128, 1], f32, tag="st")
                nc.vector.reduce_max(mx[:Pt], pf[:Pt, 0:D_BINS], axis=AX)
                negm = spool.tile([128, 1], f32, tag="st")
                nc.vector.tensor_scalar_mul(negm[:Pt], mx[:Pt], -1.0)
                nc.scalar.activation(
                    dvalb[:Pt, t, :], pf[:Pt, 0:D_BINS], ACT.Exp, bias=negm[:Pt]
                )
                sm = spool.tile([128, 1], f32, tag="st")
                nc.vector.reduce_sum(sm[:Pt], dvalb[:Pt, t, :], axis=AX)
                rc = spool.tile([128, 1], f32, tag="st")
                nc.vector.reciprocal(rc[:Pt], sm[:Pt])
                nc.vector.tensor_scalar_mul(dvalb[:Pt, t, :], dvalb[:Pt, t, :], rc[:Pt])
                nc.vector.tensor_tensor(
                    out=dvalb[:Pt, t, :], in0=dvalb[:Pt, t, :],
                    in1=mbuf[:Pt, t, :], op=OP.mult,
                )
                nc.scalar.copy(cfb[:Pt, t, :], pf[:Pt, D_BINS:NO])

            # stage D: per-column h-contraction
            pt_tiles = {}
            for a in range(CPC):
                t, po = a // 3, (a % 3) * 32
                g, q = a // 2, a % 2
                if q == 0:
                    pt_tiles[g] = ptp.tile([105, 128], f32, tag="pt", name=f"ptile{g}")
                nc.tensor.matmul(
                    pt_tiles[g][64 * q:64 * q + 41, :],
                    lhsT=dvalb[po:po + 32, t, :],
                    rhs=cfb[po:po + 32, t, :],
                    start=True,
                    stop=True,
                )
                if q == 1 or a == CPC - 1:
                    if g % 2 == 0:
                        nc.scalar.copy(tbuf[:, g, :], pt_tiles[g][:])
                    else:
                        nc.vector.tensor_copy(tbuf[:, g, :], pt_tiles[g][:])

            a2a_in = dpool.tile([SLOTS, 128], f32)
            a2a_out = dpool.tile([SLOTS, 128], f32)
            if debug:
                nc.sync.dma_start(out=dbg_t[:], in_=tbuf[:].rearrange("p g c -> p (g c)"))

            nc.sync.dma_start(
                out=a2a_in[0:SLOTS_REAL].rearrange("(g p) c -> p g c", p=105),
                in_=tbuf[:],
            )
            if SLOTS > SLOTS_REAL:
                nc.sync.dma_start(
                    out=a2a_in[SLOTS_REAL:SLOTS],
                    in_=zrows[: SLOTS - SLOTS_REAL],
                )
            nc.gpsimd.collective_compute(
                "AllToAll",
                mybir.AluOpType.bypass,
                replica_groups=[list(range(NCORES))],
                ins=[a2a_in[:].opt()],
                outs=[a2a_out[:].opt()],
            )

            if debug:
                nc.sync.dma_start(out=dbg_ain[:], in_=a2a_in[:])
                nc.sync.dma_start(out=dbg_aout[:], in_=a2a_out[:])
            ubuf = wpool.tile([128, NCHUNK, 128], f32)
            nc.sync.dma_start(
                out=ubuf[:],
                in_=a2a_out[:].rearrange("(j p) c -> p j c", p=128),
            )

            if debug:
                nc.sync.dma_start(out=dbg_u[:], in_=ubuf[:].rearrange("p j c -> p (j c)"))
            sres = wpool.tile([128, NCHUNK, 128], f32)
            for j in range(NCHUNK):
                ps = psp.tile([128, 128], f32, tag="ps", name=f"pseg{j}")
                nc.tensor.matmul(
                    ps[:], lhsT=rbuf[:, j, :], rhs=ubuf[:, j, :],
                    start=True, stop=True,
                )
                if j % 2 == 0:
                    nc.scalar.copy(sres[:, j, :], ps[:])
                else:
                    nc.vector.tensor_copy(sres[:, j, :], ps[:])

            nc.sync.dma_start(
                out=out2[:].rearrange("(j p) c -> p j c", p=128),
                in_=sres[:],
            )

    nc.compile()
    return nc


# ------------------------------ entry point -------------------------------

def kernel(**inputs) -> np.ndarray:
    global LAST_EXEC_NS, LAST_RESULTS
    from concourse import bass_utils

    pl = _make_plan(inputs)
    nc = _build_program(pl)

    in_maps = []
    for c in range(NCORES):
        in_maps.append(
            dict(
                xin=np.ascontiguousarray(pl.xin[c]),
                wt=pl.wt,
                bv=pl.bv,
                mk=np.ascontiguousarray(pl.mk[c]),
                rmat=np.ascontiguousarray(pl.rmat[c]),
            )
        )

    trace = bool(int(os.environ.get("KERNEL_TRACE", "0")))
    try:
        res = bass_utils.run_bass_kernel_spmd(
            nc, in_maps, core_ids=list(range(NCORES)), trace=trace
        )
    except ModuleNotFoundError:
        # NTFF profiling hook unavailable under this axon client; run untraced
        res = bass_utils.run_bass_kernel_spmd(
            nc, in_maps, core_ids=list(range(NCORES)), trace=False
        )
    LAST_EXEC_NS = res.exec_time_ns
    LAST_RESULTS = res

    reruns = int(os.environ.get("KERNEL_TIME_RUNS", "0"))
    if reruns > 0 and LAST_EXEC_NS is None:
        # No NTFF available: report best-of-n wall time of a cached re-run
        # (upper bound on device time; includes PJRT dispatch overhead).
        import time as _time

        best = None
        for _ in range(reruns):
            t0 = _time.perf_counter()
            res = bass_utils.run_bass_kernel_spmd(
                nc, in_maps, core_ids=list(range(NCORES)), trace=False
            )
            dt = _time.perf_counter() - t0
            best = dt if best is None else min(best, dt)
        LAST_EXEC_NS = int(best * 1e9)
        LAST_RESULTS = res

    bev = np.zeros((NSEG, 128), np.float32)
    for t in range(NCORES):
        o = res.results[t]["out2"]
        if len(pl.piece_row[t]):
            np.add.at(bev, pl.piece_rank[t], o[pl.piece_row[t]])
    final = bev.reshape(NX, NY, C_TRANS).transpose(2, 1, 0)[None]
    return np.ascontiguousarray(final.astype(np.float32))


# revision 10
# speedup vs baseline: 1.0094x; 1.0094x over previous
"""LSS (lift-splat-shoot) BEV transform kernel for 8 trn2 NeuronCores.

Pipeline (per core, SPMD single NEFF):
  stage A: feat = w_depth @ x + b  (per-pixel 1x1 conv as matmul)
  stage B: softmax over 41 depth bins, cfeat = feat[41:169]
  stage C: dvalid = depth * validity-mask (host-computed mask)
  stage D: h-contraction  T[col,d,c] = sum_h dvalid[col,h,d]*cfeat[col,h,c]
           (valid because voxel rank is h-invariant per (cam,d,w) for this
            camera geometry; host verifies and splits h-groups otherwise)
  stage E: route T rows by owning core via indirect-scatter + AllToAll
  stage F: segment-sum routed rows with one-hot matmuls into per-piece rows
Host: geometry/rank computation, routing tables, one-hot R matrices, and
final piece->voxel accumulation + layout transpose.
"""

import math
import os

import numpy as np

# ---------------- problem constants (hardcoded; must match reference) -----
OGF_H, OGF_W = 256, 704
DOWNSAMPLE = 16
FH, FW = OGF_H // DOWNSAMPLE, OGF_W // DOWNSAMPLE  # 16, 44
D_BINS = 41
C_TRANS = 128
NX, NY, NZ = 128, 128, 1
DX = np.array([0.8, 0.8, 20.0], np.float32)
BX = np.array([-50.8, -50.8, 0.0], np.float32)
NCORES = 8
CIN = 512
NSEG = NX * NY * NZ  # 16384 (B=1)

LAST_EXEC_NS = None
LAST_RESULTS = None


def _make_frustum():
    ds = np.arange(4.0, 45.0, 1.0, dtype=np.float32)[:, None, None] * np.ones(
        (1, FH, FW), np.float32
    )
    xs = np.linspace(0.0, OGF_W - 1.0, FW, dtype=np.float32)[None, None, :] * np.ones(
        (D_BINS, FH, 1), np.float32
    )
    ys = np.linspace(0.0, OGF_H - 1.0, FH, dtype=np.float32)[None, :, None] * np.ones(
        (D_BINS, 1, FW), np.float32
    )
    return np.stack([xs, ys, ds], axis=-1)  # (D, H, W, 3)


def _geometry(rots, trans, intrins, post_rots, post_trans):
    """Replicates reference get_geometry in numpy float32.
    Returns gi (B,N,D,H,W,3) int32 voxel indices and valid mask."""
    frustum = _make_frustum()
    inv_post = np.linalg.inv(post_rots.astype(np.float32)).astype(np.float32)
    inv_intr = np.linalg.inv(intrins.astype(np.float32)).astype(np.float32)
    pts = frustum[None, None] - post_trans[:, :, None, None, None, :]
    pts = np.einsum("bnij,bndhwj->bndhwi", inv_post, pts).astype(np.float32)
    pts = np.concatenate([pts[..., :2] * pts[..., 2:3], pts[..., 2:3]], axis=-1)
    combine = np.einsum("bnij,bnjk->bnik", rots, inv_intr).astype(np.float32)
    geom = (
        np.einsum("bnij,bndhwj->bndhwi", combine, pts).astype(np.float32)
        + trans[:, :, None, None, None, :]
    ).astype(np.float32)
    gi = ((geom - (BX - DX / 2.0)) / DX).astype(np.int32)
    valid = (
        (gi[..., 0] >= 0)
        & (gi[..., 0] < NX)
        & (gi[..., 1] >= 0)
        & (gi[..., 1] < NY)
        & (gi[..., 2] >= 0)
        & (gi[..., 2] < NZ)
    )
    return gi, valid


def _build_columns(gi, valid):
    """Build h-collapsed columns. Each column = (cam n, pixel w, h-mask) s.t.
    for every d the valid members share one voxel rank.
    Returns list of dicts with n, w, hmask[FH], rank[d] (int32, -1 invalid),
    mask[d, h] float32."""
    # rank per point (valid points only meaningful)
    rank = gi[..., 0].astype(np.int64) * (NY * NZ) + gi[..., 1] * NZ + gi[..., 2]
    cols = []
    B, N = gi.shape[0], gi.shape[1]
    assert B == 1
    for n in range(N):
        for w in range(FW):
            r = rank[0, n, :, :, w]  # (D, H)
            v = valid[0, n, :, :, w]  # (D, H)
            # greedy group h's so that within a group every d has <=1 rank
            groups = []  # list of (hlist, rank_per_d array)
            for h in range(FH):
                placed = False
                for hl, rpd in groups:
                    ok = True
                    for d in range(D_BINS):
                        if v[d, h] and rpd[d] >= 0 and rpd[d] != r[d, h]:
                            ok = False
                            break
                    if ok:
                        hl.append(h)
                        for d in range(D_BINS):
                            if v[d, h]:
                                rpd[d] = r[d, h]
                        placed = True
                        break
                if not placed:
                    rpd = np.full(D_BINS, -1, np.int64)
                    for d in range(D_BINS):
                        if v[d, h]:
                            rpd[d] = r[d, h]
                    groups.append(([h], rpd))
            for hl, rpd in groups:
                mask = np.zeros((D_BINS, FH), np.float32)
                for h in hl:
                    mask[:, h] = v[:, h].astype(np.float32)
                cols.append(dict(n=n, w=w, rank=rpd, mask=mask))
    return cols


def _fast_columns(gi, valid):
    """Fast path: verify rank is h-invariant per (n,d,w) among valid h's.
    Returns columns list or None if the property fails."""
    rank = gi[..., 0].astype(np.int64) * (NY * NZ) + gi[..., 1] * NZ + gi[..., 2]
    r = rank[0]  # (N, D, H, W)
    v = valid[0]
    rv = np.where(v, r, -1)
    mx = rv.max(axis=2)  # (N, D, W)
    # conflict if any valid h has rank != max
    conflict = (v & (rv != mx[:, :, None, :])).any(axis=2)  # (N, D, W)
    if conflict.any():
        return None
    cols = []
    for n in range(r.shape[0]):
        for w in range(FW):
            rpd = mx[n, :, w].copy()  # -1 where no valid h
            mask = v[n, :, :, w].astype(np.float32)  # (D, H)
            cols.append(dict(n=n, w=w, rank=rpd, mask=mask))
    return cols


class _Plan:
    pass


def _make_plan(inputs):
    x = np.asarray(inputs["x"], np.float32)
    gi, valid = _geometry(
        np.asarray(inputs["rots"], np.float32),
        np.asarray(inputs["trans"], np.float32),
        np.asarray(inputs["intrins"], np.float32),
        np.asarray(inputs["post_rots"], np.float32),
        np.asarray(inputs["post_trans"], np.float32),
    )
    cols = _fast_columns(gi, valid)
    if cols is None:
        cols = _build_columns(gi, valid)

    # pad column count to multiple of 48 (8 cores x 3 cols/tile x 2/group)
    while len(cols) % 48 != 0:
        cols.append(
            dict(n=0, w=0, rank=np.full(D_BINS, -1, np.int64),
                 mask=np.zeros((D_BINS, FH), np.float32))
        )
    NCOLS = len(cols)
    CPC = NCOLS // NCORES          # columns per core (multiple of 6)
    GROUPS = CPC // 2              # stage-D psum groups of 2 columns
    TILES = CPC // 3               # 128-partition tiles (3 cols per tile)
    PX = TILES * 128               # padded pixel partitions per core

    # ---- sortless slot scheme ----
    # packed slot space: srcslot = g*82 + 41*q + d (q=a%2, g=a//2)
    # AllToAll: dest t receives slice [t*SH,(t+1)*SH) of every source's slots
    SLOTS_REAL = 82 * GROUPS
    SH0 = (SLOTS_REAL + NCORES - 1) // NCORES
    SLOTS = ((NCORES * SH0 + 127) // 128) * 128
    SH = SLOTS // NCORES
    NCHUNK = SLOTS // 128

    # rank per (gcol, d); -1 = no contribution
    rank_of = np.full((NCOLS, D_BINS), -1, np.int64)
    for g, c in enumerate(cols):
        m_any = c["mask"].any(axis=1)
        rk = np.asarray(c["rank"])
        rank_of[g] = np.where(m_any & (rk >= 0), rk, -1)

    rmat = np.zeros((NCORES, 128, NCHUNK * 128), np.float32)
    piece_row = [[] for _ in range(NCORES)]
    piece_rank = [[] for _ in range(NCORES)]
    for t in range(NCORES):
        for j in range(NCHUNK):
            run_of = {}
            for p_loc in range(128):
                i = j * 128 + p_loc          # dest slot
                srcs = i // SH
                srcslot = t * SH + (i - srcs * SH)
                if srcslot >= SLOTS_REAL:
                    continue
                p = srcslot % 82
                g = srcslot // 82
                q = 1 if p >= 41 else 0
                d = p - 41 * q
                gcol = srcs * CPC + g * 2 + q
                rk = rank_of[gcol, d]
                if rk < 0:
                    continue
                if rk not in run_of:
                    run_of[rk] = len(run_of)
                    piece_row[t].append(j * 128 + run_of[rk])
                    piece_rank[t].append(rk)
                rmat[t, p_loc, j * 128 + run_of[rk]] = 1.0

    # ---- x_loc, mask, weights ----
    xin = np.zeros((NCORES, 4, 128, PX), np.float32)
    mk = np.zeros((NCORES, TILES, 128, D_BINS), np.float32)
    for cidx in range(NCORES):
        for a in range(CPC):
            c = cols[cidx * CPC + a]
            xcol = x[0, c["n"], :, :, c["w"]]  # (512, FH)
            base = (a // 3) * 128 + (a % 3) * 32
            xin[cidx, :, :, base:base + FH] = xcol.reshape(4, 128, FH)
            tt, po = a // 3, (a % 3) * 32
            mk[cidx, tt, po:po + FH, :] = c["mask"].T  # (FH, D)

    w_depth = np.asarray(inputs["w_depth"], np.float32)  # (169, 512)
    wt = np.ascontiguousarray(
        w_depth.T.reshape(4, 128, D_BINS + C_TRANS)
    )  # wt[k] = w_depth[:, 128k:128k+128].T
    bv = np.asarray(inputs["b_depth"], np.float32).reshape(1, D_BINS + C_TRANS)

    pl = _Plan()
    pl.NCOLS, pl.CPC, pl.GROUPS, pl.PX, pl.TILES = NCOLS, CPC, GROUPS, PX, TILES
    pl.NCHUNK, pl.SH, pl.SLOTS, pl.SLOTS_REAL = NCHUNK, SH, SLOTS, SLOTS_REAL
    pl.rmat = rmat
    pl.piece_row = [np.array(p, np.int64) for p in piece_row]
    pl.piece_rank = [np.array(p, np.int64) for p in piece_rank]
    pl.xin, pl.mk, pl.wt, pl.bv = xin, mk, wt, bv
    return pl


# ------------------------- device program ---------------------------------

def _build_program(pl):
    import concourse.bass as bass
    import concourse.mybir as mybir
    import concourse.tile as tile
    from concourse import bacc

    f32 = mybir.dt.float32
    i32 = mybir.dt.int32
    AX = mybir.AxisListType.X
    OP = mybir.AluOpType
    ACT = mybir.ActivationFunctionType

    CPC, GROUPS, PX, TILES = pl.CPC, pl.GROUPS, pl.PX, pl.TILES
    NCHUNK, SH = pl.NCHUNK, pl.SH
    NO = D_BINS + C_TRANS  # 169

    nc = bacc.Bacc("TRN2", target_bir_lowering=False, debug=False,
                   num_devices=NCORES)

    xin = nc.dram_tensor("xin", [4, 128, PX], f32, kind="ExternalInput")
    wt = nc.dram_tensor("wt", [4, 128, NO], f32, kind="ExternalInput")
    bv = nc.dram_tensor("bv", [1, NO], f32, kind="ExternalInput")
    mk = nc.dram_tensor("mk", [TILES, 128, D_BINS], f32, kind="ExternalInput")
    SLOTS, SLOTS_REAL = pl.SLOTS, pl.SLOTS_REAL
    rmat = nc.dram_tensor("rmat", [128, NCHUNK * 128], f32, kind="ExternalInput")
    out2 = nc.dram_tensor("out2", [NCHUNK * 128, 128], f32, kind="ExternalOutput")
    debug = bool(int(os.environ.get("KERNEL_DEBUG", "0")))
    if debug:
        dbg_t = nc.dram_tensor("dbg_t", [105, GROUPS * 128], f32, kind="ExternalOutput")
        dbg_ain = nc.dram_tensor("dbg_ain", [SLOTS, 128], f32, kind="ExternalOutput")
        dbg_aout = nc.dram_tensor("dbg_aout", [SLOTS, 128], f32, kind="ExternalOutput")
        dbg_u = nc.dram_tensor("dbg_u", [128, NCHUNK * 128], f32, kind="ExternalOutput")

    with tile.TileContext(nc) as tc:
        with (
            tc.tile_pool(name="const", bufs=1) as cpool,
            tc.tile_pool(name="work", bufs=1) as wpool,
            tc.tile_pool(name="stats", bufs=4) as spool,
            tc.tile_pool(name="pf", bufs=2, space="PSUM") as pfp,
            tc.tile_pool(name="pt", bufs=4, space="PSUM") as ptp,
            tc.tile_pool(name="ps", bufs=2, space="PSUM") as psp,
            tc.tile_pool(name="dram", bufs=1, space="DRAM") as dpool,
        ):
            xbuf = cpool.tile([128, 4, PX], f32)
            wbuf = cpool.tile([128, 4, NO], f32)
            bbuf = cpool.tile([1, NO], f32)
            mbuf = cpool.tile([128, TILES, D_BINS], f32)
            rbuf = cpool.tile([128, NCHUNK, 128], f32)
            onesb = cpool.tile([1, PX], f32)

            for k in range(4):
                nc.sync.dma_start(out=xbuf[:, k, :], in_=xin[k])
                nc.sync.dma_start(out=wbuf[:, k, :], in_=wt[k])
            nc.sync.dma_start(out=bbuf[:], in_=bv[:])
            for t in range(TILES):
                nc.sync.dma_start(out=mbuf[:, t, :], in_=mk[t])
            nc.sync.dma_start(
                out=rbuf[:].rearrange("p j c -> p (j c)"), in_=rmat[:]
            )
            nc.vector.memset(onesb[:], 1.0)

            dvalb = wpool.tile([128, TILES, D_BINS], f32)
            cfb = wpool.tile([128, TILES, C_TRANS], f32)
            tbuf = wpool.tile([105, GROUPS, 128], f32)
            zrows = cpool.tile([64, 128], f32)
            nc.vector.memset(zrows[:], 0.0)

            for t in range(TILES):
                Pt = 128
                pf = pfp.tile([128, NO], f32)
                for k in range(4):
                    nc.tensor.matmul(
                        pf[:Pt],
                        lhsT=xbuf[:, k, t * 128:t * 128 + Pt],
                        rhs=wbuf[:, k, :],
                        start=(k == 0),
                        stop=False,
                    )
                nc.tensor.matmul(
                    pf[:Pt],
                    lhsT=onesb[:1, t * 128:t * 128 + Pt],
                    rhs=bbuf[:1, :],
                    start=False,
                    stop=True,
                )
                mx = spool.tile([128, 1], f32, tag="st")
                nc.vector.reduce_max(mx[:Pt], pf[:Pt, 0:D_BINS], axis=AX)
                negm = spool.tile([128, 1], f32, tag="st")
                nc.vector.tensor_scalar_mul(negm[:Pt], mx[:Pt], -1.0)
                nc.scalar.activation(
                    dvalb[:Pt, t, :], pf[:Pt, 0:D_BINS], ACT.Exp, bias=negm[:Pt]
                )
                sm = spool.tile([128, 1], f32, tag="st")
                nc.vector.reduce_sum(sm[:Pt], dvalb[:Pt, t, :], axis=AX)
                rc = spool.tile([128, 1], f32, tag="st")
                nc.vector.reciprocal(rc[:Pt], sm[:Pt])
                nc.vector.tensor_scalar_mul(dvalb[:Pt, t, :], dvalb[:Pt, t, :], rc[:Pt])
                nc.vector.tensor_tensor(
                    out=dvalb[:Pt, t, :], in0=dvalb[:Pt, t, :],
                    in1=mbuf[:Pt, t, :], op=OP.mult,
                )
                nc.scalar.copy(cfb[:Pt, t, :], pf[:Pt, D_BINS:NO])

            # stage D: per-column h-contraction
            pt_tiles = {}
            for a in range(CPC):
                t, po = a // 3, (a % 3) * 32
                g, q = a // 2, a % 2
                if q == 0:
                    pt_tiles[g] = ptp.tile([105, 128], f32, tag="pt", name=f"ptile{g}")
                nc.tensor.matmul(
                    pt_tiles[g][64 * q:64 * q + 41, :],
                    lhsT=dvalb[po:po + 32, t, :],
                    rhs=cfb[po:po + 32, t, :],
                    start=True,
                    stop=True,
                )
                if q == 1 or a == CPC - 1:
                    if g % 2 == 0:
                        nc.scalar.copy(tbuf[:, g, :], pt_tiles[g][:])
                    else:
                        nc.vector.tensor_copy(tbuf[:, g, :], pt_tiles[g][:])

            a2a_in = dpool.tile([SLOTS, 128], f32)
            a2a_out = dpool.tile([SLOTS, 128], f32)
            if debug:
                nc.sync.dma_start(out=dbg_t[:], in_=tbuf[:].rearrange("p g c -> p (g c)"))

            a2a_view = a2a_in[0:SLOTS_REAL].rearrange("(g p) c -> p g c", p=82)
            nc.sync.dma_start(out=a2a_view[0:41], in_=tbuf[0:41, :, :])
            nc.sync.dma_start(out=a2a_view[41:82], in_=tbuf[64:105, :, :])
            if SLOTS > SLOTS_REAL:
                nc.sync.dma_start(
                    out=a2a_in[SLOTS_REAL:SLOTS],
                    in_=zrows[: SLOTS - SLOTS_REAL],
                )
            nc.gpsimd.collective_compute(
                "AllToAll",
                mybir.AluOpType.bypass,
                replica_groups=[list(range(NCORES))],
                ins=[a2a_in[:].opt()],
                outs=[a2a_out[:].opt()],
            )

            if debug:
                nc.sync.dma_start(out=dbg_ain[:], in_=a2a_in[:])
                nc.sync.dma_start(out=dbg_aout[:], in_=a2a_out[:])
            ubuf = wpool.tile([128, NCHUNK, 128], f32)
            nc.sync.dma_start(
                out=ubuf[:],
                in_=a2a_out[:].rearrange("(j p) c -> p j c", p=128),
            )

            if debug:
                nc.sync.dma_start(out=dbg_u[:], in_=ubuf[:].rearrange("p j c -> p (j c)"))
            sres = wpool.tile([128, NCHUNK, 128], f32)
            for j in range(NCHUNK):
                ps = psp.tile([128, 128], f32, tag="ps", name=f"pseg{j}")
                nc.tensor.matmul(
                    ps[:], lhsT=rbuf[:, j, :], rhs=ubuf[:, j, :],
                    start=True, stop=True,
                )
                if j % 2 == 0:
                    nc.scalar.copy(sres[:, j, :], ps[:])
                else:
                    nc.vector.tensor_copy(sres[:, j, :], ps[:])

            nc.sync.dma_start(
                out=out2[:].rearrange("(j p) c -> p j c", p=128),
                in_=sres[:],
            )

    nc.compile()
    return nc


# ------------------------------ entry point -------------------------------

def kernel(**inputs) -> np.ndarray:
    global LAST_EXEC_NS, LAST_RESULTS
    from concourse import bass_utils

    pl = _make_plan(inputs)
    nc = _build_program(pl)

    in_maps = []
    for c in range(NCORES):
        in_maps.append(
            dict(
                xin=np.ascontiguousarray(pl.xin[c]),
                wt=pl.wt,
                bv=pl.bv,
                mk=np.ascontiguousarray(pl.mk[c]),
                rmat=np.ascontiguousarray(pl.rmat[c]),
            )
        )

    trace = bool(int(os.environ.get("KERNEL_TRACE", "0")))
    try:
        res = bass_utils.run_bass_kernel_spmd(
            nc, in_maps, core_ids=list(range(NCORES)), trace=trace
        )
    except ModuleNotFoundError:
        # NTFF profiling hook unavailable under this axon client; run untraced
        res = bass_utils.run_bass_kernel_spmd(
            nc, in_maps, core_ids=list(range(NCORES)), trace=False
        )
    LAST_EXEC_NS = res.exec_time_ns
    LAST_RESULTS = res

    reruns = int(os.environ.get("KERNEL_TIME_RUNS", "0"))
    if reruns > 0 and LAST_EXEC_NS is None:
        # No NTFF available: report best-of-n wall time of a cached re-run
        # (upper bound on device time; includes PJRT dispatch overhead).
        import time as _time

        best = None
        for _ in range(reruns):
            t0 = _time.perf_counter()
            res = bass_utils.run_bass_kernel_spmd(
                nc, in_maps, core_ids=list(range(NCORES)), trace=False
            )
            dt = _time.perf_counter() - t0
            best = dt if best is None else min(best, dt)
        LAST_EXEC_NS = int(best * 1e9)
        LAST_RESULTS = res

    bev = np.zeros((NSEG, 128), np.float32)
    for t in range(NCORES):
        o = res.results[t]["out2"]
        if len(pl.piece_row[t]):
            np.add.at(bev, pl.piece_rank[t], o[pl.piece_row[t]])
    final = bev.reshape(NX, NY, C_TRANS).transpose(2, 1, 0)[None]
    return np.ascontiguousarray(final.astype(np.float32))
